# revision 31
# baseline (speedup 1.0000x reference)
"""Distributed MemoryEfficientAttention for 8 TRN2 NeuronCores.

Reference computation (B=2, N=2048, C=1024, H=16, D=64):
    qkv = x @ qkv_w.T + qkv_b                  [B,N,3C]
    q, k, v = split; q *= D**-0.5
    q, k = rope(q), rope(k)                    (interleaved pairs, halves concat)
    attn = softmax(q @ k.T / sqrt(D))
    out = (attn @ v) reshaped                  [B,N,C]
    y = out @ proj_w.T + proj_b

Sharding: 8 cores = batch (2) x head-groups (4 groups of 4 heads).
Each core computes its 4 heads end-to-end plus the partial output
projection; the host sums the 4 partials per batch and adds biases.

Per-core kernel layout notes:
  * All matmuls run in bf16 with f32 PSUM accumulation.
  * q/k weight columns are pre-permuted on the host into (evens, odds)
    pair order and q is pre-scaled by 1/D, so RoPE becomes
    rot = qk * cs + swap32(qk) * ss with cs/ss [128, N] tables and
    swap32 done by an SBUF->SBUF DMA (the only partition-crossing move).
  * Scores are computed transposed (keys on partitions). Softmax skips
    the max-subtraction (scores are ~N(0, 0.05) by construction), and
    row sums ride for free in the attn@V matmul via a ones column in
    the stationary [V | 1] operand.
  * Normalization: reciprocal of the sums row, partition-broadcast via
    a K=1 matmul against a ones vector, multiplied into O^T.
"""

import sys

if "/opt/trn_rl_repo" not in sys.path:
    sys.path.append("/opt/trn_rl_repo")

import numpy as np
import ml_dtypes

import concourse.bacc as bacc
import concourse.tile as tile
import concourse.mybir as mybir
from concourse.bass_utils import run_bass_kernel_spmd

BF16 = mybir.dt.bfloat16
F32 = mybir.dt.float32
AF = mybir.ActivationFunctionType

B, N, C = 2, 2048, 1024
H, D = 16, 64
HL = 4            # local heads per core
P = 128
CCH = C // P      # 8 contraction chunks for C
NQB = 512         # query block
NKC = N // P      # 16 key chunks
VROW = 130        # per-pair v_sb cols: [V_even(64) | 1 | V_odd(64) | 1]

_NC_CACHE = {}


def _build_nc(with_qk_bias: bool):
    nc = bacc.Bacc("TRN2", target_bir_lowering=False)

    xT_d = nc.dram_tensor("xT", [C, N], BF16, kind="ExternalInput")
    wqk_d = nc.dram_tensor("wqk", [C, 4 * P], BF16, kind="ExternalInput")
    wv_d = nc.dram_tensor("wv", [C, HL * D], BF16, kind="ExternalInput")
    wp_d = nc.dram_tensor("wp", [HL * D, C], BF16, kind="ExternalInput")
    cs_d = nc.dram_tensor("cs", [P, N], F32, kind="ExternalInput")
    ss_d = nc.dram_tensor("ss", [P, N], F32, kind="ExternalInput")
    bqk_d = nc.dram_tensor("bqk", [P, 4], F32, kind="ExternalInput")
    y_d = nc.dram_tensor("y", [N, C], F32, kind="ExternalOutput")

    with tile.TileContext(nc) as tc:
        with tc.tile_pool(name="singles", bufs=1) as singles, \
             tc.tile_pool(name="swp", bufs=2) as swp_pool, \
             tc.tile_pool(name="qkn", bufs=3) as qkn_pool, \
             tc.tile_pool(name="rtmp", bufs=3) as rtmp_pool, \
             tc.tile_pool(name="attn", bufs=10) as attn_pool, \
             tc.tile_pool(name="osb", bufs=6) as osb_pool, \
             tc.tile_pool(name="rbp", bufs=4) as rb_pool, \
             tc.tile_pool(name="ysb", bufs=6) as ysb_pool, \
             tc.tile_pool(name="ps", bufs=2, space="PSUM") as ps_pool, \
             tc.tile_pool(name="po", bufs=4, space="PSUM") as po_pool:

            # ---- persistent tiles -------------------------------------
            xT = singles.tile([P, CCH, N], BF16, tag="xT")
            wqk = singles.tile([P, CCH, 4 * P], BF16, tag="wqk")
            wv = singles.tile([P, CCH, HL * D], BF16, tag="wv")
            wp = singles.tile([P, 2, C], BF16, tag="wp")
            cs = singles.tile([P, N], F32, tag="cs")
            ss = singles.tile([P, N], F32, tag="ss")
            bqk = singles.tile([P, 4], F32, tag="bqk")
            ones = singles.tile([P, 64], BF16, tag="ones")
            vsb = [singles.tile([P, 2 * VROW], BF16, tag=f"vsb{k}", name=f"vsb{k}")
                   for k in range(NKC)]
            rot = [[singles.tile([P, NQB], BF16, tag=f"rot{m}{b}", name=f"rot{m}{b}")
                    for b in range(4)] for m in range(4)]
            otn = [[singles.tile([P, NQB], BF16, tag=f"otn{c}{q}", name=f"otn{c}{q}")
                    for q in range(4)] for c in range(2)]

            nc.sync.dma_start(out=wqk[:], in_=wqk_d[:].rearrange("(c p) o -> p c o", p=P))
            for c in range(CCH):
                nc.sync.dma_start(
                    out=xT[:, c],
                    in_=xT_d[:].rearrange("(c p) n -> p c n", p=P)[:, c],
                )
            nc.sync.dma_start(out=cs[:], in_=cs_d[:])
            nc.sync.dma_start(out=ss[:], in_=ss_d[:])
            nc.sync.dma_start(out=wv[:], in_=wv_d[:].rearrange("(c p) o -> p c o", p=P))
            nc.sync.dma_start(out=wp[:], in_=wp_d[:].rearrange("(c p) o -> p c o", p=P))
            if with_qk_bias:
                nc.sync.dma_start(out=bqk[:], in_=bqk_d[:])
            nc.vector.memset(ones[:], 1.0)
            for k in range(NKC):
                nc.gpsimd.memset(vsb[k][:], 0.0)
                for pair in range(2):
                    nc.gpsimd.memset(vsb[k][:, pair * VROW + 64:pair * VROW + 65], 1.0)
                    nc.gpsimd.memset(vsb[k][:, pair * VROW + 129:pair * VROW + 130], 1.0)

            # ---- emission helpers -------------------------------------
            # chunk m: 0 = q pair0, 1 = q pair1, 2 = k pair0, 3 = k pair1
            # chunk rows: [head_even (32 ev + 32 od) | head_odd (32 ev + 32 od)]
            def emit_qk_tail1(m, nb, ps, st):
                qn = qkn_pool.tile([P, NQB], F32, tag="qkn", name=f"qn{m}{nb}")
                if with_qk_bias:
                    nc.vector.tensor_scalar_add(
                        out=qn[:], in0=ps[:], scalar1=bqk[:, m:m + 1],
                    )
                else:
                    nc.vector.tensor_copy(out=qn[:], in_=ps[:])
                # swap32: (0-31,32-63,64-95,96-127) -> (32-63,0-31,96-127,64-95)
                sw = swp_pool.tile([P, NQB], F32, tag="swp", name=f"sw{m}{nb}")
                for dst, src in ((0, 32), (32, 0), (64, 96), (96, 64)):
                    nc.sync.dma_start(
                        out=sw[dst:dst + 32, :], in_=qn[src:src + 32, :]
                    )
                nsl = slice(nb * NQB, (nb + 1) * NQB)
                t1 = rtmp_pool.tile([P, NQB], F32, tag="rtmp", name=f"t1{m}{nb}")
                nc.vector.tensor_mul(out=t1[:], in0=qn[:], in1=cs[:, nsl])
                st["qn"], st["sw"], st["t1"] = qn, sw, t1

            def emit_qk_tail2(m, nb, st):
                nsl = slice(nb * NQB, (nb + 1) * NQB)
                t2 = rtmp_pool.tile([P, NQB], F32, tag="rtmp", name=f"t2{m}{nb}")
                nc.vector.tensor_mul(out=t2[:], in0=st["sw"][:], in1=ss[:, nsl])
                nc.vector.tensor_add(out=rot[m][nb][:], in0=st["t1"][:], in1=t2[:])

            def emit_qk(m):
                for nb in range(4):
                    nsl = slice(nb * NQB, (nb + 1) * NQB)
                    ps2 = ps_pool.tile([P, 2, NQB], F32, tag="ps", name=f"qk{m}{nb}")
                    ps = ps2[:, 0]
                    for c in range(CCH):
                        nc.tensor.matmul(
                            ps[:],
                            wqk[:, c, m * P:(m + 1) * P],
                            xT[:, c, nsl],
                            start=(c == 0),
                            stop=(c == CCH - 1),
                        )
                    st = {}
                    emit_qk_tail1(m, nb, ps, st)
                    emit_qk_tail2(m, nb, st)

            def qk_fillers(m):
                """Filler closures: 8 matmuls + rope tail per nb chunk."""
                st = {}
                fl = []
                for nb in range(4):
                    for c in range(CCH):
                        def f(m=m, nb=nb, c=c):
                            if c == 0:
                                st[nb] = po_pool.tile(
                                    [P, NQB], F32, tag="po", name=f"qf{m}{nb}"
                                )
                            nc.tensor.matmul(
                                st[nb][:],
                                wqk[:, c, m * P:(m + 1) * P],
                                xT[:, c, nb * NQB:(nb + 1) * NQB],
                                start=(c == 0),
                                stop=(c == CCH - 1),
                            )
                        f.pe = True
                        fl.append(f)
                    tst = {}
                    def fin1(m=m, nb=nb, tst=tst):
                        emit_qk_tail1(m, nb, st[nb], tst)
                    fin1.pe = False
                    fl.append(fin1)
                    def fin2(m=m, nb=nb, tst=tst):
                        emit_qk_tail2(m, nb, tst)
                    fin2.pe = False
                    fl.append(fin2)
                return fl

            def proj_fillers(qb):
                st = {}
                fl = []
                for nsq in range(4):
                    ns = qb * 4 + nsq
                    for cb in range(2):
                        for dc in range(2):
                            def f(qb=qb, nsq=nsq, ns=ns, cb=cb, dc=dc):
                                if dc == 0:
                                    st[(ns, cb)] = po_pool.tile(
                                        [P, NQB], F32, tag="po",
                                        name=f"pyf{ns}{cb}",
                                    )
                                nc.tensor.matmul(
                                    st[(ns, cb)][:],
                                    otn[dc][qb][:, nsq * P:(nsq + 1) * P],
                                    wp[:, dc, cb * NQB:(cb + 1) * NQB],
                                    start=(dc == 0),
                                    stop=(dc == 1),
                                )
                            f.pe = True
                            fl.append(f)
                        def fin(ns=ns, cb=cb):
                            py = st[(ns, cb)]
                            ys = ysb_pool.tile([P, NQB], F32, tag="ysb",
                                               name=f"ys{ns}{cb}")
                            nc.vector.tensor_copy(out=ys[:], in_=py[:])
                            nc.sync.dma_start(
                                out=y_d[ns * P:(ns + 1) * P,
                                        cb * NQB:(cb + 1) * NQB],
                                in_=ys[:],
                            )
                        fin.pe = False
                        fl.append(fin)
                return fl

            def emit_v():
                for kc in range(NKC):
                    ps2 = ps_pool.tile([P, 2, NQB], F32, tag="ps", name=f"v{kc}")
                    ps = ps2[:, 0]
                    for c in range(CCH):
                        nc.tensor.matmul(
                            ps[:, :HL * D],
                            xT[:, c, kc * P:(kc + 1) * P],
                            wv[:, c, :],
                            start=(c == 0),
                            stop=(c == CCH - 1),
                        )
                    for pair in range(2):
                        base = pair * VROW
                        nc.vector.tensor_copy(
                            out=vsb[kc][:, base:base + 64],
                            in_=ps[:, pair * 128:pair * 128 + 64],
                        )
                        nc.vector.tensor_copy(
                            out=vsb[kc][:, base + 65:base + 129],
                            in_=ps[:, pair * 128 + 64:pair * 128 + 128],
                        )

            def pop_fillers(fillers, npe):
                done = 0
                while fillers and done < npe:
                    f = fillers.popleft()
                    f()
                    if f.pe:
                        done += 1

            pending = []

            def emit_attn_block(pair, qb, fillers=None):
                rq = rot[pair][qb]
                vbase = pair * VROW
                oA = po_pool.tile([P, NQB], F32, tag="po", name=f"oA{pair}{qb}")
                oB = po_pool.tile([P, NQB], F32, tag="po", name=f"oB{pair}{qb}")
                LAG = 4
                atiles = {}
                for step in range(NKC + LAG):
                    if step < NKC:
                        kc = step
                        rk = rot[2 + pair][kc // 4]
                        ksl = slice((kc % 4) * P, (kc % 4 + 1) * P)
                        sAB = ps_pool.tile([P, 2, NQB], F32, tag="ps",
                                           name=f"s{pair}{qb}{kc}")
                        nc.tensor.matmul(
                            sAB[:, 0], rk[0:64, ksl], rq[0:64, :],
                            start=True, stop=True,
                        )
                        nc.tensor.matmul(
                            sAB[:, 1], rk[64:128, ksl], rq[64:128, :],
                            start=True, stop=True,
                        )
                        aAB = attn_pool.tile([P, 2, NQB], BF16, tag="at",
                                             name=f"a{pair}{qb}{kc}")
                        nc.scalar.activation(out=aAB[:], in_=sAB[:], func=AF.Exp)
                        atiles[kc] = aAB
                    if step >= LAG:
                        kc = step - LAG
                        aAB = atiles.pop(kc)
                        # [V | 1] stationary: rows 0-63 = O^T, row 64 = sums
                        nc.tensor.matmul(
                            oA[0:65, :], vsb[kc][:, vbase:vbase + 65], aAB[:, 0],
                            start=(kc == 0), stop=(kc == NKC - 1),
                        )
                        nc.tensor.matmul(
                            oB[0:65, :],
                            vsb[kc][:, vbase + 65:vbase + 130], aAB[:, 1],
                            start=(kc == 0), stop=(kc == NKC - 1),
                        )
                    if step == 8 and pending:
                        pending.pop(0)()
                    if fillers is not None and 4 <= step <= 13:
                        pop_fillers(fillers, 2 if step >= 6 else 1)
                oAs = osb_pool.tile([65, NQB], F32, tag="os", name=f"oAs{pair}{qb}")
                oBs = osb_pool.tile([65, NQB], F32, tag="os", name=f"oBs{pair}{qb}")
                nc.vector.tensor_copy(out=oAs[:], in_=oA[0:65, :])
                nc.vector.tensor_copy(out=oBs[:], in_=oB[0:65, :])
                rbA = rb_pool.tile([64, NQB], F32, tag="rb", name=f"rbA{pair}{qb}")
                rbB = rb_pool.tile([64, NQB], F32, tag="rb", name=f"rbB{pair}{qb}")
                nc.sync.dma_start(
                    out=rbA[:], in_=oAs[64:65, None, :].to_broadcast((1, 64, NQB))
                )
                nc.sync.dma_start(
                    out=rbB[:], in_=oBs[64:65, None, :].to_broadcast((1, 64, NQB))
                )

                def normalize_tail(pair=pair, qb=qb, oAs=oAs, oBs=oBs,
                                   rbA=rbA, rbB=rbB):
                    nc.vector.reciprocal_approx_fast(out=rbA[:], in_=rbA[:])
                    nc.vector.reciprocal_approx_fast(out=rbB[:], in_=rbB[:])
                    nc.vector.tensor_mul(
                        out=otn[pair][qb][0:64, :], in0=oAs[0:64, :], in1=rbA[:]
                    )
                    oto = ysb_pool.tile([64, NQB], BF16, tag="oto",
                                        name=f"ot{pair}{qb}")
                    nc.vector.tensor_mul(out=oto[:], in0=oBs[0:64, :], in1=rbB[:])
                    nc.gpsimd.dma_start(
                        out=otn[pair][qb][64:128, :], in_=oto[:]
                    )
                pending.append(normalize_tail)

            def emit_proj(qb):
                for nsq in range(4):
                    ns = qb * 4 + nsq
                    for cb in range(2):
                        py = po_pool.tile([P, NQB], F32, tag="po",
                                          name=f"py{ns}{cb}")
                        for dc in range(2):
                            nc.tensor.matmul(
                                py[:],
                                otn[dc][:, ns * P:(ns + 1) * P],
                                wp[:, dc, cb * NQB:(cb + 1) * NQB],
                                start=(dc == 0),
                                stop=(dc == 1),
                            )
                        ys = ysb_pool.tile([P, NQB], F32, tag="ysb",
                                           name=f"ys{ns}{cb}")
                        nc.vector.tensor_copy(out=ys[:], in_=py[:])
                        nc.sync.dma_start(
                            out=y_d[ns * P:(ns + 1) * P, cb * NQB:(cb + 1) * NQB],
                            in_=ys[:],
                        )

            # ---- interleaved emission ---------------------------------
            from collections import deque
            emit_qk(0)
            emit_qk(2)
            emit_v()
            fillers = deque(qk_fillers(3) + qk_fillers(1))
            for qb in range(4):
                emit_attn_block(0, qb, fillers)
            emit_attn_block(1, 0, fillers)
            emit_attn_block(1, 1, fillers)
            fillers.extend(proj_fillers(0))
            emit_attn_block(1, 2, fillers)
            fillers.extend(proj_fillers(1))
            emit_attn_block(1, 3, fillers)
            for t in pending:
                t()
            pending.clear()
            pop_fillers(fillers, 1000)
            for qb in (2, 3):
                for f in proj_fillers(qb):
                    f()

    nc.compile()
    return nc


def _rope_tables():
    inv_freq = 1.0 / (10000.0 ** (np.arange(0, D, 2, dtype=np.float64) / D))
    t = np.arange(N, dtype=np.float64)
    freqs = np.outer(t, inv_freq)                       # [N, 32]
    cosT = np.cos(freqs).T.astype(np.float32)           # [32, N]
    sinT = np.sin(freqs).T.astype(np.float32)
    cs = np.concatenate([cosT, cosT, cosT, cosT], axis=0)       # [128, N]
    ss = np.concatenate([-sinT, sinT, -sinT, sinT], axis=0)     # [128, N]
    return np.ascontiguousarray(cs), np.ascontiguousarray(ss)


def _pair_perm():
    # per 64-dim head block: evens then odds
    return np.concatenate([np.arange(0, D, 2), np.arange(1, D, 2)])


def prepare_core_inputs(x, qkv_w, qkv_b, proj_w, cs, ss):
    """Build the 8 per-core input dicts."""
    perm = _pair_perm()
    bf = ml_dtypes.bfloat16
    in_maps = []
    group_cache = {}
    for core in range(8):
        b, g = divmod(core, 4)
        if g not in group_cache:
            heads = [4 * g + i for i in range(HL)]
            wq_cols = []
            wk_cols = []
            bq = np.zeros((4, P), np.float32)
            for i, h in enumerate(heads):
                rq = qkv_w[h * D:(h + 1) * D][perm] * (1.0 / D)     # [64, C]
                rk = qkv_w[C + h * D:C + (h + 1) * D][perm]
                wq_cols.append(rq.T)
                wk_cols.append(rk.T)
                chunk, half = divmod(i, 2)
                bq[chunk, half * 64:(half + 1) * 64] = \
                    qkv_b[h * D:(h + 1) * D][perm] * (1.0 / D)
                bq[2 + chunk, half * 64:(half + 1) * 64] = \
                    qkv_b[C + h * D:C + (h + 1) * D][perm]
            wqk = np.concatenate(wq_cols + wk_cols, axis=1)         # [C, 512]
            wv = np.concatenate(
                [qkv_w[2 * C + h * D:2 * C + (h + 1) * D].T for h in heads],
                axis=1,
            )                                                       # [C, 256]
            wp = np.concatenate(
                [proj_w[:, h * D:(h + 1) * D].T for h in heads], axis=0
            )                                                       # [256, C]
            group_cache[g] = (
                np.ascontiguousarray(wqk).astype(bf),
                np.ascontiguousarray(wv).astype(bf),
                np.ascontiguousarray(wp).astype(bf),
                np.ascontiguousarray(bq.T),                         # [128, 4]
            )
        wqk, wv, wp, bqk = group_cache[g]
        xT = np.ascontiguousarray(x[b].T).astype(bf)                # [C, N]
        in_maps.append({
            "xT": xT, "wqk": wqk, "wv": wv, "wp": wp,
            "cs": cs, "ss": ss, "bqk": bqk,
        })
    return in_maps


_TRACE = False
LAST_RESULT = None


def kernel(x, qkv_w, qkv_b, proj_w, proj_b):
    global LAST_RESULT
    x = np.asarray(x, dtype=np.float32)
    qkv_w = np.asarray(qkv_w, dtype=np.float32)
    qkv_b = np.asarray(qkv_b, dtype=np.float32)
    proj_w = np.asarray(proj_w, dtype=np.float32)
    proj_b = np.asarray(proj_b, dtype=np.float32)

    with_qk_bias = bool(np.any(qkv_b[:2 * C]))
    key = with_qk_bias
    if key not in _NC_CACHE:
        _NC_CACHE[key] = _build_nc(with_qk_bias)
    nc = _NC_CACHE[key]

    cs, ss = _rope_tables()
    in_maps = prepare_core_inputs(x, qkv_w, qkv_b, proj_w, cs, ss)
    res = run_bass_kernel_spmd(nc, in_maps, core_ids=list(range(8)), trace=_TRACE)
    LAST_RESULT = res

    # host reduce: sum 4 head-group partials per batch, add exact bias terms
    const = proj_w @ qkv_b[2 * C:] + proj_b                         # [C]
    y = np.empty((B, N, C), np.float32)
    for b in range(B):
        acc = res.results[4 * b]["y"].astype(np.float32).copy()
        for g in range(1, 4):
            acc += res.results[4 * b + g]["y"]
        y[b] = acc + const
    return y


# revision 32
# speedup vs baseline: 1.0215x; 1.0215x over previous
"""Distributed MemoryEfficientAttention for 8 TRN2 NeuronCores.

Reference computation (B=2, N=2048, C=1024, H=16, D=64):
    qkv = x @ qkv_w.T + qkv_b                  [B,N,3C]
    q, k, v = split; q *= D**-0.5
    q, k = rope(q), rope(k)                    (interleaved pairs, halves concat)
    attn = softmax(q @ k.T / sqrt(D))
    out = (attn @ v) reshaped                  [B,N,C]
    y = out @ proj_w.T + proj_b

Sharding: 8 cores = batch (2) x head-groups (4 groups of 4 heads).
Each core computes its 4 heads end-to-end plus the partial output
projection; the host sums the 4 partials per batch and adds biases.

Per-core kernel layout notes:
  * All matmuls run in bf16 with f32 PSUM accumulation.
  * q/k weight columns are pre-permuted on the host into (evens, odds)
    pair order and q is pre-scaled by 1/D, so RoPE becomes
    rot = qk * cs + swap32(qk) * ss with cs/ss [128, N] tables and
    swap32 done by an SBUF->SBUF DMA (the only partition-crossing move).
  * Scores are computed transposed (keys on partitions). Softmax skips
    the max-subtraction (scores are ~N(0, 0.05) by construction), and
    row sums ride for free in the attn@V matmul via a ones column in
    the stationary [V | 1] operand.
  * Normalization: reciprocal of the sums row, partition-broadcast via
    a K=1 matmul against a ones vector, multiplied into O^T.
"""

import sys

if "/opt/trn_rl_repo" not in sys.path:
    sys.path.append("/opt/trn_rl_repo")

import numpy as np
import ml_dtypes

import concourse.bacc as bacc
import concourse.tile as tile
import concourse.mybir as mybir
from concourse.bass_utils import run_bass_kernel_spmd

BF16 = mybir.dt.bfloat16
F32 = mybir.dt.float32
AF = mybir.ActivationFunctionType

B, N, C = 2, 2048, 1024
H, D = 16, 64
HL = 4            # local heads per core
P = 128
CCH = C // P      # 8 contraction chunks for C
NQB = 512         # query block
NKC = N // P      # 16 key chunks
VROW = 130        # per-pair v_sb cols: [V_even(64) | 1 | V_odd(64) | 1]

_NC_CACHE = {}


def _build_nc(with_qk_bias: bool):
    nc = bacc.Bacc("TRN2", target_bir_lowering=False)

    xT_d = nc.dram_tensor("xT", [C, N], BF16, kind="ExternalInput")
    wqk_d = nc.dram_tensor("wqk", [C, 4 * P], BF16, kind="ExternalInput")
    wv_d = nc.dram_tensor("wv", [C, HL * D], BF16, kind="ExternalInput")
    wp_d = nc.dram_tensor("wp", [HL * D, C], BF16, kind="ExternalInput")
    cs_d = nc.dram_tensor("cs", [P, N], F32, kind="ExternalInput")
    ss_d = nc.dram_tensor("ss", [P, N], F32, kind="ExternalInput")
    bqk_d = nc.dram_tensor("bqk", [P, 4], F32, kind="ExternalInput")
    y_d = nc.dram_tensor("y", [N, C], F32, kind="ExternalOutput")

    with tile.TileContext(nc) as tc:
        with tc.tile_pool(name="singles", bufs=1) as singles, \
             tc.tile_pool(name="swp", bufs=2) as swp_pool, \
             tc.tile_pool(name="qkn", bufs=3) as qkn_pool, \
             tc.tile_pool(name="rtmp", bufs=3) as rtmp_pool, \
             tc.tile_pool(name="attn", bufs=10) as attn_pool, \
             tc.tile_pool(name="osb", bufs=6) as osb_pool, \
             tc.tile_pool(name="rbp", bufs=4) as rb_pool, \
             tc.tile_pool(name="ysb", bufs=6) as ysb_pool, \
             tc.tile_pool(name="ps", bufs=2, space="PSUM") as ps_pool, \
             tc.tile_pool(name="po", bufs=4, space="PSUM") as po_pool:

            # ---- persistent tiles -------------------------------------
            xT = singles.tile([P, CCH, N], BF16, tag="xT")
            wqk = singles.tile([P, CCH, 4 * P], BF16, tag="wqk")
            wv = singles.tile([P, CCH, HL * D], BF16, tag="wv")
            wp = singles.tile([P, 2, C], BF16, tag="wp")
            cs = singles.tile([P, N], F32, tag="cs")
            ss = singles.tile([P, N], F32, tag="ss")
            bqk = singles.tile([P, 4], F32, tag="bqk")
            ones = singles.tile([P, 64], BF16, tag="ones")
            vsb = [singles.tile([P, 2 * VROW], BF16, tag=f"vsb{k}", name=f"vsb{k}")
                   for k in range(NKC)]
            rot = [[singles.tile([P, NQB], BF16, tag=f"rot{m}{b}", name=f"rot{m}{b}")
                    for b in range(4)] for m in range(4)]
            otn = [[singles.tile([P, NQB], BF16, tag=f"otn{c}{q}", name=f"otn{c}{q}")
                    for q in range(4)] for c in range(2)]

            nc.sync.dma_start(out=wqk[:], in_=wqk_d[:].rearrange("(c p) o -> p c o", p=P))
            for c in range(CCH):
                nc.sync.dma_start(
                    out=xT[:, c],
                    in_=xT_d[:].rearrange("(c p) n -> p c n", p=P)[:, c],
                )
            nc.sync.dma_start(out=cs[:], in_=cs_d[:])
            nc.sync.dma_start(out=ss[:], in_=ss_d[:])
            nc.sync.dma_start(out=wv[:], in_=wv_d[:].rearrange("(c p) o -> p c o", p=P))
            nc.sync.dma_start(out=wp[:], in_=wp_d[:].rearrange("(c p) o -> p c o", p=P))
            if with_qk_bias:
                nc.sync.dma_start(out=bqk[:], in_=bqk_d[:])
            nc.vector.memset(ones[:], 1.0)
            for k in range(NKC):
                nc.gpsimd.memset(vsb[k][:], 0.0)
                for pair in range(2):
                    nc.gpsimd.memset(vsb[k][:, pair * VROW + 64:pair * VROW + 65], 1.0)
                    nc.gpsimd.memset(vsb[k][:, pair * VROW + 129:pair * VROW + 130], 1.0)

            # ---- emission helpers -------------------------------------
            # chunk m: 0 = q pair0, 1 = q pair1, 2 = k pair0, 3 = k pair1
            # chunk rows: [head_even (32 ev + 32 od) | head_odd (32 ev + 32 od)]
            def emit_qk_tail1(m, nb, ps, st):
                qn = qkn_pool.tile([P, NQB], F32, tag="qkn", name=f"qn{m}{nb}")
                if with_qk_bias:
                    nc.vector.tensor_scalar_add(
                        out=qn[:], in0=ps[:], scalar1=bqk[:, m:m + 1],
                    )
                else:
                    nc.vector.tensor_copy(out=qn[:], in_=ps[:])
                # swap32: (0-31,32-63,64-95,96-127) -> (32-63,0-31,96-127,64-95)
                sw = swp_pool.tile([P, NQB], F32, tag="swp", name=f"sw{m}{nb}")
                for dst, src in ((0, 32), (32, 0), (64, 96), (96, 64)):
                    nc.sync.dma_start(
                        out=sw[dst:dst + 32, :], in_=qn[src:src + 32, :]
                    )
                nsl = slice(nb * NQB, (nb + 1) * NQB)
                t1 = rtmp_pool.tile([P, NQB], F32, tag="rtmp", name=f"t1{m}{nb}")
                nc.vector.tensor_mul(out=t1[:], in0=qn[:], in1=cs[:, nsl])
                st["qn"], st["sw"], st["t1"] = qn, sw, t1

            def emit_qk_tail2(m, nb, st):
                nsl = slice(nb * NQB, (nb + 1) * NQB)
                t2 = rtmp_pool.tile([P, NQB], F32, tag="rtmp", name=f"t2{m}{nb}")
                nc.vector.tensor_mul(out=t2[:], in0=st["sw"][:], in1=ss[:, nsl])
                nc.vector.tensor_add(out=rot[m][nb][:], in0=st["t1"][:], in1=t2[:])

            def emit_qk(m):
                for nb in range(4):
                    nsl = slice(nb * NQB, (nb + 1) * NQB)
                    ps2 = ps_pool.tile([P, 2, NQB], F32, tag="ps", name=f"qk{m}{nb}")
                    ps = ps2[:, 0]
                    for c in range(CCH):
                        nc.tensor.matmul(
                            ps[:],
                            wqk[:, c, m * P:(m + 1) * P],
                            xT[:, c, nsl],
                            start=(c == 0),
                            stop=(c == CCH - 1),
                        )
                    st = {}
                    emit_qk_tail1(m, nb, ps, st)
                    emit_qk_tail2(m, nb, st)

            def qk_fillers(m):
                """Filler closures: 8 matmuls + rope tail per nb chunk."""
                st = {}
                fl = []
                for nb in range(4):
                    for c in range(CCH):
                        def f(m=m, nb=nb, c=c):
                            if c == 0:
                                st[nb] = po_pool.tile(
                                    [P, NQB], F32, tag="po", name=f"qf{m}{nb}"
                                )
                            nc.tensor.matmul(
                                st[nb][:],
                                wqk[:, c, m * P:(m + 1) * P],
                                xT[:, c, nb * NQB:(nb + 1) * NQB],
                                start=(c == 0),
                                stop=(c == CCH - 1),
                            )
                        f.pe = True
                        fl.append(f)
                    tst = {}
                    def fin1(m=m, nb=nb, tst=tst):
                        emit_qk_tail1(m, nb, st[nb], tst)
                    fin1.pe = False
                    fl.append(fin1)
                    def fin2(m=m, nb=nb, tst=tst):
                        emit_qk_tail2(m, nb, tst)
                    fin2.pe = False
                    fl.append(fin2)
                return fl

            def proj_fillers(qb):
                st = {}
                fl = []
                for nsq in range(4):
                    ns = qb * 4 + nsq
                    for cb in range(2):
                        for dc in range(2):
                            def f(qb=qb, nsq=nsq, ns=ns, cb=cb, dc=dc):
                                if dc == 0:
                                    st[(ns, cb)] = po_pool.tile(
                                        [P, NQB], F32, tag="po",
                                        name=f"pyf{ns}{cb}",
                                    )
                                nc.tensor.matmul(
                                    st[(ns, cb)][:],
                                    otn[dc][qb][:, nsq * P:(nsq + 1) * P],
                                    wp[:, dc, cb * NQB:(cb + 1) * NQB],
                                    start=(dc == 0),
                                    stop=(dc == 1),
                                )
                            f.pe = True
                            fl.append(f)
                        def fin(ns=ns, cb=cb):
                            py = st[(ns, cb)]
                            ys = ysb_pool.tile([P, NQB], F32, tag="ysb",
                                               name=f"ys{ns}{cb}")
                            nc.vector.tensor_copy(out=ys[:], in_=py[:])
                            nc.sync.dma_start(
                                out=y_d[ns * P:(ns + 1) * P,
                                        cb * NQB:(cb + 1) * NQB],
                                in_=ys[:],
                            )
                        fin.pe = False
                        fl.append(fin)
                return fl

            def emit_v():
                for kc in range(NKC):
                    ps2 = ps_pool.tile([P, 2, NQB], F32, tag="ps", name=f"v{kc}")
                    ps = ps2[:, 0]
                    for c in range(CCH):
                        nc.tensor.matmul(
                            ps[:, :HL * D],
                            xT[:, c, kc * P:(kc + 1) * P],
                            wv[:, c, :],
                            start=(c == 0),
                            stop=(c == CCH - 1),
                        )
                    for pair in range(2):
                        base = pair * VROW
                        nc.vector.tensor_copy(
                            out=vsb[kc][:, base:base + 64],
                            in_=ps[:, pair * 128:pair * 128 + 64],
                        )
                        nc.vector.tensor_copy(
                            out=vsb[kc][:, base + 65:base + 129],
                            in_=ps[:, pair * 128 + 64:pair * 128 + 128],
                        )

            def pop_fillers(fillers, npe):
                done = 0
                while fillers and done < npe:
                    f = fillers.popleft()
                    f()
                    if f.pe:
                        done += 1

            pending = []

            def emit_attn_block(pair, qb, fillers=None):
                rq = rot[pair][qb]
                vbase = pair * VROW
                oA = po_pool.tile([P, NQB], F32, tag="po", name=f"oA{pair}{qb}")
                oB = po_pool.tile([P, NQB], F32, tag="po", name=f"oB{pair}{qb}")
                LAG = 3
                atiles = {}
                for step in range(NKC + LAG):
                    if step < NKC:
                        kc = step
                        rk = rot[2 + pair][kc // 4]
                        ksl = slice((kc % 4) * P, (kc % 4 + 1) * P)
                        sAB = ps_pool.tile([P, 2, NQB], F32, tag="ps",
                                           name=f"s{pair}{qb}{kc}")
                        nc.tensor.matmul(
                            sAB[:, 0], rk[0:64, ksl], rq[0:64, :],
                            start=True, stop=True,
                        )
                        nc.tensor.matmul(
                            sAB[:, 1], rk[64:128, ksl], rq[64:128, :],
                            start=True, stop=True,
                        )
                        aAB = attn_pool.tile([P, 2, NQB], BF16, tag="at",
                                             name=f"a{pair}{qb}{kc}")
                        nc.scalar.activation(out=aAB[:], in_=sAB[:], func=AF.Exp)
                        atiles[kc] = aAB
                    if step >= LAG:
                        kc = step - LAG
                        aAB = atiles.pop(kc)
                        # [V | 1] stationary: rows 0-63 = O^T, row 64 = sums
                        nc.tensor.matmul(
                            oA[0:65, :], vsb[kc][:, vbase:vbase + 65], aAB[:, 0],
                            start=(kc == 0), stop=(kc == NKC - 1),
                        )
                        nc.tensor.matmul(
                            oB[0:65, :],
                            vsb[kc][:, vbase + 65:vbase + 130], aAB[:, 1],
                            start=(kc == 0), stop=(kc == NKC - 1),
                        )
                    if step == 8 and pending:
                        pending.pop(0)()
                    if fillers is not None and 4 <= step <= 13:
                        pop_fillers(fillers, 2 if step >= 6 else 1)
                oAs = osb_pool.tile([65, NQB], F32, tag="os", name=f"oAs{pair}{qb}")
                oBs = osb_pool.tile([65, NQB], F32, tag="os", name=f"oBs{pair}{qb}")
                nc.vector.tensor_copy(out=oAs[:], in_=oA[0:65, :])
                nc.vector.tensor_copy(out=oBs[:], in_=oB[0:65, :])
                rbA = rb_pool.tile([64, NQB], F32, tag="rb", name=f"rbA{pair}{qb}")
                rbB = rb_pool.tile([64, NQB], F32, tag="rb", name=f"rbB{pair}{qb}")
                nc.sync.dma_start(
                    out=rbA[:], in_=oAs[64:65, None, :].to_broadcast((1, 64, NQB))
                )
                nc.sync.dma_start(
                    out=rbB[:], in_=oBs[64:65, None, :].to_broadcast((1, 64, NQB))
                )

                def normalize_tail(pair=pair, qb=qb, oAs=oAs, oBs=oBs,
                                   rbA=rbA, rbB=rbB):
                    nc.vector.reciprocal_approx_fast(out=rbA[:], in_=rbA[:])
                    nc.vector.reciprocal_approx_fast(out=rbB[:], in_=rbB[:])
                    nc.vector.tensor_mul(
                        out=otn[pair][qb][0:64, :], in0=oAs[0:64, :], in1=rbA[:]
                    )
                    oto = ysb_pool.tile([64, NQB], BF16, tag="oto",
                                        name=f"ot{pair}{qb}")
                    nc.vector.tensor_mul(out=oto[:], in0=oBs[0:64, :], in1=rbB[:])
                    nc.gpsimd.dma_start(
                        out=otn[pair][qb][64:128, :], in_=oto[:]
                    )
                pending.append(normalize_tail)

            def emit_proj(qb):
                for nsq in range(4):
                    ns = qb * 4 + nsq
                    for cb in range(2):
                        py = po_pool.tile([P, NQB], F32, tag="po",
                                          name=f"py{ns}{cb}")
                        for dc in range(2):
                            nc.tensor.matmul(
                                py[:],
                                otn[dc][:, ns * P:(ns + 1) * P],
                                wp[:, dc, cb * NQB:(cb + 1) * NQB],
                                start=(dc == 0),
                                stop=(dc == 1),
                            )
                        ys = ysb_pool.tile([P, NQB], F32, tag="ysb",
                                           name=f"ys{ns}{cb}")
                        nc.vector.tensor_copy(out=ys[:], in_=py[:])
                        nc.sync.dma_start(
                            out=y_d[ns * P:(ns + 1) * P, cb * NQB:(cb + 1) * NQB],
                            in_=ys[:],
                        )

            # ---- interleaved emission ---------------------------------
            from collections import deque
            emit_qk(0)
            emit_qk(2)
            emit_v()
            fillers = deque(qk_fillers(3) + qk_fillers(1))
            for qb in range(4):
                emit_attn_block(0, qb, fillers)
            emit_attn_block(1, 0, fillers)
            emit_attn_block(1, 1, fillers)
            fillers.extend(proj_fillers(0))
            emit_attn_block(1, 2, fillers)
            fillers.extend(proj_fillers(1))
            emit_attn_block(1, 3, fillers)
            for t in pending:
                t()
            pending.clear()
            pop_fillers(fillers, 1000)
            for qb in (2, 3):
                for f in proj_fillers(qb):
                    f()

    nc.compile()
    return nc


def _rope_tables():
    inv_freq = 1.0 / (10000.0 ** (np.arange(0, D, 2, dtype=np.float64) / D))
    t = np.arange(N, dtype=np.float64)
    freqs = np.outer(t, inv_freq)                       # [N, 32]
    cosT = np.cos(freqs).T.astype(np.float32)           # [32, N]
    sinT = np.sin(freqs).T.astype(np.float32)
    cs = np.concatenate([cosT, cosT, cosT, cosT], axis=0)       # [128, N]
    ss = np.concatenate([-sinT, sinT, -sinT, sinT], axis=0)     # [128, N]
    return np.ascontiguousarray(cs), np.ascontiguousarray(ss)


def _pair_perm():
    # per 64-dim head block: evens then odds
    return np.concatenate([np.arange(0, D, 2), np.arange(1, D, 2)])


def prepare_core_inputs(x, qkv_w, qkv_b, proj_w, cs, ss):
    """Build the 8 per-core input dicts."""
    perm = _pair_perm()
    bf = ml_dtypes.bfloat16
    in_maps = []
    group_cache = {}
    for core in range(8):
        b, g = divmod(core, 4)
        if g not in group_cache:
            heads = [4 * g + i for i in range(HL)]
            wq_cols = []
            wk_cols = []
            bq = np.zeros((4, P), np.float32)
            for i, h in enumerate(heads):
                rq = qkv_w[h * D:(h + 1) * D][perm] * (1.0 / D)     # [64, C]
                rk = qkv_w[C + h * D:C + (h + 1) * D][perm]
                wq_cols.append(rq.T)
                wk_cols.append(rk.T)
                chunk, half = divmod(i, 2)
                bq[chunk, half * 64:(half + 1) * 64] = \
                    qkv_b[h * D:(h + 1) * D][perm] * (1.0 / D)
                bq[2 + chunk, half * 64:(half + 1) * 64] = \
                    qkv_b[C + h * D:C + (h + 1) * D][perm]
            wqk = np.concatenate(wq_cols + wk_cols, axis=1)         # [C, 512]
            wv = np.concatenate(
                [qkv_w[2 * C + h * D:2 * C + (h + 1) * D].T for h in heads],
                axis=1,
            )                                                       # [C, 256]
            wp = np.concatenate(
                [proj_w[:, h * D:(h + 1) * D].T for h in heads], axis=0
            )                                                       # [256, C]
            group_cache[g] = (
                np.ascontiguousarray(wqk).astype(bf),
                np.ascontiguousarray(wv).astype(bf),
                np.ascontiguousarray(wp).astype(bf),
                np.ascontiguousarray(bq.T),                         # [128, 4]
            )
        wqk, wv, wp, bqk = group_cache[g]
        xT = np.ascontiguousarray(x[b].T).astype(bf)                # [C, N]
        in_maps.append({
            "xT": xT, "wqk": wqk, "wv": wv, "wp": wp,
            "cs": cs, "ss": ss, "bqk": bqk,
        })
    return in_maps


_TRACE = False
LAST_RESULT = None


def kernel(x, qkv_w, qkv_b, proj_w, proj_b):
    global LAST_RESULT
    x = np.asarray(x, dtype=np.float32)
    qkv_w = np.asarray(qkv_w, dtype=np.float32)
    qkv_b = np.asarray(qkv_b, dtype=np.float32)
    proj_w = np.asarray(proj_w, dtype=np.float32)
    proj_b = np.asarray(proj_b, dtype=np.float32)

    with_qk_bias = bool(np.any(qkv_b[:2 * C]))
    key = with_qk_bias
    if key not in _NC_CACHE:
        _NC_CACHE[key] = _build_nc(with_qk_bias)
    nc = _NC_CACHE[key]

    cs, ss = _rope_tables()
    in_maps = prepare_core_inputs(x, qkv_w, qkv_b, proj_w, cs, ss)
    res = run_bass_kernel_spmd(nc, in_maps, core_ids=list(range(8)), trace=_TRACE)
    LAST_RESULT = res

    # host reduce: sum 4 head-group partials per batch, add exact bias terms
    const = proj_w @ qkv_b[2 * C:] + proj_b                         # [C]
    y = np.empty((B, N, C), np.float32)
    for b in range(B):
        acc = res.results[4 * b]["y"].astype(np.float32).copy()
        for g in range(1, 4):
            acc += res.results[4 * b + g]["y"]
        y[b] = acc + const
    return y
